# revision 5
# baseline (speedup 1.0000x reference)
"""Causal multi-head attention (S=2048, B=2, H=16, D=128, fp32) on 8 trn2 cores.

Sharding: the 32 (batch, head) pairs are split 4-per-core (tensor parallel on
heads). Each core runs a flash-attention-style kernel in the "S^T layout":

  For a query chunk c (512 wide) and key block j (128 wide):
    S^T[k, q] = (K_j^T)^T-matmul: lhsT = K^T[d, k_j], rhs = Q^T[d, q_c]  (PE)
    P^T = exp(S^T * 1/sqrt(D))                                          (ACT)
    causal mask via affine_select (keep where q >= k, else 0)           (DVE)
    ctx^T[d, q_c] += V_j^T-matmul: lhsT = V[k_j, d], rhs = P^T          (PE)
    l[q_c]       += ones-matmul:   lhsT = 1[k_j, 1], rhs = P^T          (PE)

Host pre-transposes Q/K to [d, s] per head so no on-chip transposes are
needed anywhere, and does the final divide ctx/l (mathematically identical
to normalizing P before the V matmul).
"""

import sys

if "/opt/trn_rl_repo" not in sys.path:
    sys.path.insert(0, "/opt/trn_rl_repo")

import numpy as np

S, B, H, D = 2048, 2, 16, 128
N_CORES = 8
HPC = (B * H) // N_CORES  # head-slices per core = 4
QCH = 512  # query chunk width (max fp32 moving dim / one PSUM bank)
NCH = S // QCH  # 4 chunks
NKB = S // 128  # 16 key blocks
SCALE = 1.0 / float(np.sqrt(D))

# fp32r: PE reduced-precision fp32 mode, 4x faster at moving dim >= 256.
MM_DTYPE = "float32"

_compiled = None


def _build():
    import concourse.tile as tile
    from concourse import bacc, mybir

    mm_dt = getattr(mybir.dt, MM_DTYPE)

    nc = bacc.Bacc("TRN2", target_bir_lowering=False, debug=False)
    qT = nc.dram_tensor("qT", [HPC, D, S], mybir.dt.float32, kind="ExternalInput").ap()
    kT = nc.dram_tensor("kT", [HPC, D, S], mybir.dt.float32, kind="ExternalInput").ap()
    v = nc.dram_tensor("v", [HPC, S, D], mybir.dt.float32, kind="ExternalInput").ap()
    out = nc.dram_tensor(
        "out", [HPC, D, S], mybir.dt.float32, kind="ExternalOutput"
    ).ap()
    lsum = nc.dram_tensor(
        "lsum", [HPC, S], mybir.dt.float32, kind="ExternalOutput"
    ).ap()

    with tile.TileContext(nc) as tc:
        with (
            tc.tile_pool(name="const", bufs=1) as const_pool,
            tc.tile_pool(name="io", bufs=2) as io_pool,
            tc.tile_pool(name="p", bufs=3) as p_pool,
            tc.tile_pool(name="psum_s", bufs=2, space="PSUM") as psum_s,
            tc.tile_pool(name="psum_ctx", bufs=2, space="PSUM") as psum_ctx,
            tc.tile_pool(name="psum_l", bufs=2, space="PSUM") as psum_l,
        ):
            ones_s = const_pool.tile([128, 1], mybir.dt.float32)
            nc.vector.memset(ones_s[:], 1.0)

            for h in range(HPC):
                qT_s = io_pool.tile([128, S], mybir.dt.float32, tag="qT_s")
                nc.sync.dma_start(qT_s[:], qT[h])
                kT_s = io_pool.tile([128, S], mybir.dt.float32, tag="kT_s")
                nc.sync.dma_start(kT_s[:], kT[h])
                v_s = io_pool.tile([128, NKB * 128], mybir.dt.float32, tag="v_s")
                nc.sync.dma_start(
                    v_s[:].rearrange("p (j d) -> p j d", d=128),
                    v[h].rearrange("(j p) d -> p j d", p=128),
                )

                for c in range(NCH):
                    ctx_c = psum_ctx.tile([128, QCH], mybir.dt.float32, tag="ctx")
                    l_c = psum_l.tile([1, QCH], mybir.dt.float32, tag="l")
                    jmax = 4 * c + 3
                    for j in range(jmax + 1):
                        s_t = psum_s.tile([128, QCH], mybir.dt.float32, tag="s")
                        nc.tensor.matmul(
                            s_t[:],
                            kT_s[:, j * 128 : (j + 1) * 128].bitcast(mm_dt),
                            qT_s[:, c * QCH : (c + 1) * QCH].bitcast(mm_dt),
                            start=True,
                            stop=True,
                        )
                        p_t = p_pool.tile([128, QCH], mybir.dt.float32, tag="p")
                        nc.scalar.activation(
                            p_t[:],
                            s_t[:],
                            mybir.ActivationFunctionType.Exp,
                            scale=SCALE,
                        )
                        if j >= 4 * c:
                            # keep where q_global >= k_global:
                            # iota = (c*QCH + col) - (j*128 + part) >= 0
                            nc.gpsimd.affine_select(
                                p_t[:],
                                p_t[:],
                                pattern=[[1, QCH]],
                                base=c * QCH - j * 128,
                                channel_multiplier=-1,
                                compare_op=mybir.AluOpType.is_ge,
                                fill=0.0,
                            )
                        nc.tensor.matmul(
                            ctx_c[:],
                            v_s[:, j * 128 : (j + 1) * 128].bitcast(mm_dt),
                            p_t[:].bitcast(mm_dt),
                            start=(j == 0),
                            stop=(j == jmax),
                            skip_group_check=True,
                        )
                        nc.tensor.matmul(
                            l_c[:],
                            ones_s[:].bitcast(mm_dt),
                            p_t[:].bitcast(mm_dt),
                            start=(j == 0),
                            stop=(j == jmax),
                            skip_group_check=True,
                        )
                    o_t = p_pool.tile([128, QCH], mybir.dt.float32, tag="o")
                    nc.vector.tensor_copy(o_t[:], ctx_c[:])
                    nc.sync.dma_start(out[h][:, c * QCH : (c + 1) * QCH], o_t[:])
                    lo_t = p_pool.tile([1, QCH], mybir.dt.float32, tag="lo")
                    nc.vector.tensor_copy(lo_t[:], l_c[:])
                    nc.sync.dma_start(
                        lsum[h : h + 1, c * QCH : (c + 1) * QCH], lo_t[:]
                    )

    nc.compile()
    return nc


def _get_compiled():
    global _compiled
    if _compiled is None:
        _compiled = _build()
    return _compiled


def _run(query_layer, key_layer, value_layer, attention_mask=None, trace=False):
    from concourse import bass_utils

    nc = _get_compiled()

    q = np.asarray(query_layer, dtype=np.float32)
    k = np.asarray(key_layer, dtype=np.float32)
    v = np.asarray(value_layer, dtype=np.float32)

    # [S,B,H,D] -> [BH, D, S] for q/k, [BH, S, D] for v
    qT_all = np.ascontiguousarray(q.transpose(1, 2, 3, 0).reshape(B * H, D, S))
    kT_all = np.ascontiguousarray(k.transpose(1, 2, 3, 0).reshape(B * H, D, S))
    v_all = np.ascontiguousarray(v.transpose(1, 2, 0, 3).reshape(B * H, S, D))

    in_maps = [
        {
            "qT": qT_all[c * HPC : (c + 1) * HPC],
            "kT": kT_all[c * HPC : (c + 1) * HPC],
            "v": v_all[c * HPC : (c + 1) * HPC],
        }
        for c in range(N_CORES)
    ]
    res = bass_utils.run_bass_kernel_spmd(
        nc, in_maps, list(range(N_CORES)), trace=trace
    )

    ctxT = np.concatenate(
        [res.results[c]["out"] for c in range(N_CORES)], axis=0
    )  # [BH, D, S]
    l = np.concatenate(
        [res.results[c]["lsum"] for c in range(N_CORES)], axis=0
    )  # [BH, S]
    ctxT = ctxT / l[:, None, :]
    # [BH, D, S] -> [S, B, H*D]
    full = ctxT.reshape(B, H, D, S).transpose(3, 0, 1, 2).reshape(S, B, H * D)
    return np.ascontiguousarray(full.astype(np.float32)), res


def kernel(query_layer, key_layer, value_layer, attention_mask=None):
    out, _ = _run(query_layer, key_layer, value_layer, attention_mask)
    return out


# revision 11
# speedup vs baseline: 2.8392x; 2.8392x over previous
"""Causal multi-head attention (S=2048, B=2, H=16, D=128, fp32) on 8 trn2 cores.

Sharding: the 32 (batch, head) pairs are split 4-per-core (tensor parallel on
heads). Each core runs a flash-attention-style kernel in the "S^T layout":

  For a query chunk c (512 wide) and key block j (128 wide):
    S^T[k, q] = (K_j^T)^T-matmul: lhsT = K^T[d, k_j], rhs = Q^T[d, q_c]  (PE)
    P^T = exp(S^T * 1/sqrt(D))                                          (ACT)
    causal mask via affine_select (keep where q >= k, else 0)           (DVE)
    ctx^T[d, q_c] += V_j^T-matmul: lhsT = V[k_j, d], rhs = P^T          (PE)
    l[q_c]       += ones-matmul:   lhsT = 1[k_j, 1], rhs = P^T          (PE)

Host pre-transposes Q/K to [d, s] per head so no on-chip transposes are
needed anywhere, and does the final divide ctx/l (mathematically identical
to normalizing P before the V matmul).
"""

import sys

if "/opt/trn_rl_repo" not in sys.path:
    sys.path.insert(0, "/opt/trn_rl_repo")

import numpy as np

S, B, H, D = 2048, 2, 16, 128
N_CORES = 8
HPC = (B * H) // N_CORES  # head-slices per core = 4
QCH = 512  # query chunk width (max fp32 moving dim / one PSUM bank)
NCH = S // QCH  # 4 chunks
NKB = S // 128  # 16 key blocks
SCALE = 1.0 / float(np.sqrt(D))

# fp32r: PE reduced-precision fp32 mode, 4x faster at moving dim >= 256.
MM_DTYPE = "float32r"

_compiled = None


def _build():
    import concourse.tile as tile
    from concourse import bacc, mybir

    mm_dt = getattr(mybir.dt, MM_DTYPE)

    nc = bacc.Bacc("TRN2", target_bir_lowering=False, debug=False)
    qT = nc.dram_tensor("qT", [HPC, D, S], mm_dt, kind="ExternalInput").ap()
    kT = nc.dram_tensor("kT", [HPC, D, S], mm_dt, kind="ExternalInput").ap()
    v = nc.dram_tensor("v", [HPC, S, D], mm_dt, kind="ExternalInput").ap()
    out = nc.dram_tensor(
        "out", [HPC, D, S], mybir.dt.float32, kind="ExternalOutput"
    ).ap()
    lsum = nc.dram_tensor(
        "lsum", [HPC, S], mybir.dt.float32, kind="ExternalOutput"
    ).ap()

    with tile.TileContext(nc) as tc:
        with (
            tc.tile_pool(name="const", bufs=1) as const_pool,
            tc.tile_pool(name="io", bufs=2) as io_pool,
            tc.tile_pool(name="p", bufs=3) as p_pool,
            tc.tile_pool(name="psum_s", bufs=2, space="PSUM") as psum_s,
            tc.tile_pool(name="psum_ctx", bufs=2, space="PSUM") as psum_ctx,
            tc.tile_pool(name="psum_l", bufs=2, space="PSUM") as psum_l,
        ):
            ones_f32 = const_pool.tile([128, 1], mybir.dt.float32)
            nc.vector.memset(ones_f32[:], 1.0)
            ones_s = const_pool.tile([128, 1], mm_dt)
            nc.vector.tensor_copy(ones_s[:], ones_f32[:])

            for h in range(HPC):
                qT_s = io_pool.tile([128, S], mm_dt, tag="qT_s")
                nc.sync.dma_start(qT_s[:], qT[h])
                kT_s = io_pool.tile([128, S], mm_dt, tag="kT_s")
                nc.sync.dma_start(kT_s[:], kT[h])
                v_s = io_pool.tile([128, NKB * 128], mm_dt, tag="v_s")
                nc.sync.dma_start(
                    v_s[:].rearrange("p (j d) -> p j d", d=128),
                    v[h].rearrange("(j p) d -> p j d", p=128),
                )

                for c in range(NCH):
                    ctx_c = psum_ctx.tile([128, QCH], mybir.dt.float32, tag="ctx")
                    l_c = psum_l.tile([1, QCH], mybir.dt.float32, tag="l")
                    jmax = 4 * c + 3
                    for j in range(jmax + 1):
                        s_t = psum_s.tile([128, QCH], mybir.dt.float32, tag="s")
                        nc.tensor.matmul(
                            s_t[:],
                            kT_s[:, j * 128 : (j + 1) * 128],
                            qT_s[:, c * QCH : (c + 1) * QCH],
                            start=True,
                            stop=True,
                        )
                        p_t = p_pool.tile([128, QCH], mm_dt, tag="p")
                        nc.scalar.activation(
                            p_t[:],
                            s_t[:],
                            mybir.ActivationFunctionType.Exp,
                            scale=SCALE,
                        )
                        if j >= 4 * c:
                            # keep where q_global >= k_global:
                            # iota = (c*QCH + col) - (j*128 + part) >= 0
                            nc.gpsimd.affine_select(
                                p_t[:],
                                p_t[:],
                                pattern=[[1, QCH]],
                                base=c * QCH - j * 128,
                                channel_multiplier=-1,
                                compare_op=mybir.AluOpType.is_ge,
                                fill=0.0,
                            )
                        nc.tensor.matmul(
                            ctx_c[:],
                            v_s[:, j * 128 : (j + 1) * 128],
                            p_t[:],
                            start=(j == 0),
                            stop=(j == jmax),
                            skip_group_check=True,
                        )
                        nc.tensor.matmul(
                            l_c[:],
                            ones_s[:],
                            p_t[:],
                            start=(j == 0),
                            stop=(j == jmax),
                            skip_group_check=True,
                        )
                    o_t = p_pool.tile([128, QCH], mybir.dt.float32, tag="o")
                    nc.vector.tensor_copy(o_t[:], ctx_c[:])
                    nc.sync.dma_start(out[h][:, c * QCH : (c + 1) * QCH], o_t[:])
                    lo_t = p_pool.tile([1, QCH], mybir.dt.float32, tag="lo")
                    nc.vector.tensor_copy(lo_t[:], l_c[:])
                    nc.sync.dma_start(
                        lsum[h : h + 1, c * QCH : (c + 1) * QCH], lo_t[:]
                    )

    nc.compile()
    return nc


def _get_compiled():
    global _compiled
    if _compiled is None:
        _compiled = _build()
    return _compiled


def _run(query_layer, key_layer, value_layer, attention_mask=None, trace=False):
    from concourse import bass_utils

    nc = _get_compiled()

    q = np.asarray(query_layer, dtype=np.float32)
    k = np.asarray(key_layer, dtype=np.float32)
    v = np.asarray(value_layer, dtype=np.float32)

    # [S,B,H,D] -> [BH, D, S] for q/k, [BH, S, D] for v
    qT_all = np.ascontiguousarray(q.transpose(1, 2, 3, 0).reshape(B * H, D, S))
    kT_all = np.ascontiguousarray(k.transpose(1, 2, 3, 0).reshape(B * H, D, S))
    v_all = np.ascontiguousarray(v.transpose(1, 2, 0, 3).reshape(B * H, S, D))

    in_maps = [
        {
            "qT": qT_all[c * HPC : (c + 1) * HPC],
            "kT": kT_all[c * HPC : (c + 1) * HPC],
            "v": v_all[c * HPC : (c + 1) * HPC],
        }
        for c in range(N_CORES)
    ]
    res = bass_utils.run_bass_kernel_spmd(
        nc, in_maps, list(range(N_CORES)), trace=trace
    )

    ctxT = np.concatenate(
        [res.results[c]["out"] for c in range(N_CORES)], axis=0
    )  # [BH, D, S]
    l = np.concatenate(
        [res.results[c]["lsum"] for c in range(N_CORES)], axis=0
    )  # [BH, S]
    ctxT = ctxT / l[:, None, :]
    # [BH, D, S] -> [S, B, H*D]
    full = ctxT.reshape(B, H, D, S).transpose(3, 0, 1, 2).reshape(S, B, H * D)
    return np.ascontiguousarray(full.astype(np.float32)), res


def kernel(query_layer, key_layer, value_layer, attention_mask=None):
    out, _ = _run(query_layer, key_layer, value_layer, attention_mask)
    return out
